# revision 1
# baseline (speedup 1.0000x reference)
"""DOSACon loss on 8 Trainium2 NeuronCores (Bass/Tile, SPMD data-parallel).

Math: the [N,N] broadcast in the localization term is rank-1 separable --
  mean(dw * hw * (1-ciou)^g / (area+eps)) over [N,N]
    = (sum_i dw_i*hw_i*(1-ciou_i)^g) * (sum_j 1/(area_j+eps)) / N^2
so each core computes partial sums over its 1024-row shard of the N=8192
boxes.  The 100-pair contrastive term is gathered on host (pure data
movement) and evaluated on-device in a packed 9th column / pair lane.

Device layout per core: one packed [128, 592] f32 input buffer
  cols   0: 36  P2  = px|py|tx|ty   (4 blocks of 9: 8 shard cols + 1 pair col)
  cols  36: 72  WH  = pw|ph|tw|th   (same block structure)
  cols  72: 80  density shard
  cols  80:336  gathered embeddings i (rows 100.. zero padded)
  cols 336:592  gathered embeddings j
Output per core: [128, 3] partials (col0 = loc numerator terms, col1 =
inverse-area terms, col2 = per-pair masked hinge^2; rows 100.. of col2 are
pad garbage and ignored on host).

Engine plan: DVE carries the serial CIoU chain; Pool (gpsimd) takes the
independent sub-chains (enclosing-box square, center distance, embedding
diff); ACT ops are grouped so only two activation tables load (Arctan+Sigmoid
share "sigmoid_and_others", loaded during the preamble; the late Sqrt
loads "sqrt_and_others" once).
"""

from contextlib import ExitStack

import numpy as np

N_CORES = 8
N = 8192
NS = N // N_CORES      # 1024 boxes per core
PPART = 128            # SBUF partitions
FREE = NS // PPART     # 8 shard columns
W = FREE + 1           # 9 = shard columns + 1 pair column
D = 256
NPAIR = 100

GAMMA = 2.5
ALPHA_D = 1.2
DELTA = 1.0
TAU = 0.3
LAMBDA_C = 0.5
EPS = 1e-7

_BUILT = None          # cached nc across calls
LAST_RESULT = None     # last BassKernelResults (for profiling in test.py)


def _build_nc():
    import concourse.bacc as bacc
    import concourse.mybir as mybir
    import concourse.tile as tile
    from concourse.tile import add_dep_helper

    dt = mybir.dt.float32
    A = mybir.AluOpType
    AF = mybir.ActivationFunctionType
    AX = mybir.AxisListType

    nc = bacc.Bacc("TRN2", target_bir_lowering=False, debug=False,
                   num_devices=N_CORES)
    buf_d = nc.dram_tensor("buf", [PPART, 592], dt, kind="ExternalInput")
    out_d = nc.dram_tensor("out", [PPART, 3], dt, kind="ExternalOutput")

    with tile.TileContext(nc) as tc, ExitStack() as ctx:
        pool = ctx.enter_context(tc.tile_pool(name="p", bufs=1))

        def T(n, tag):
            return pool.tile([PPART, n], dt, name=tag, tag=tag)

        # two tiles so box math only waits on the small first DMA, not on
        # the embeddings transfer (Tile tracks deps per tile)
        bufA = T(80, "bufA")
        bufB = T(512, "bufB")
        nc.sync.dma_start(bufA[:], buf_d.ap()[:, 0:80])
        nc.sync.dma_start(bufB[:], buf_d.ap()[:, 80:592])

        P2 = bufA[:, 0:36]      # px|py|tx|ty
        WH = bufA[:, 36:72]     # pw|ph|tw|th
        dn = bufA[:, 72:80]
        ei = bufB[:, 0:256]
        ej = bufB[:, 256:512]
        whr = WH.rearrange("p (a b) -> p a b", b=W)
        w_in = whr[:, 0::2, :]  # pw|tw  [128,2,9]
        h_in = whr[:, 1::2, :]  # ph|th  [128,2,9]

        def r2(ap):             # view a [128,18] tile as [128,2,9]
            return ap.rearrange("p (a b) -> p a b", b=W)

        V, S, G = nc.vector, nc.scalar, nc.gpsimd

        # === DVE: aspect-ratio chain first so Arctan is the first ACT op
        # (its table then loads during the preamble; Sigmoid shares it and
        # only the late Sqrt needs a second table load).
        # HW Arctan only covers [-pi/2, pi/2]; ratios are in (0, inf), so
        # use arctan(x) = pi/4 + arctan((x-1)/(x+1)) -- the pi/4 shift
        # cancels in the arctan difference, and (x-1)/(x+1) is in (-1, 1).
        rh = T(18, "rh")
        V.reciprocal(r2(rh[:]), h_in)
        rat = T(18, "rat")
        V.tensor_tensor(r2(rat[:]), w_in, r2(rh[:]), A.mult)
        zd = T(18, "zd")
        V.tensor_scalar_add(zd[:], rat[:], 1.0)
        rzd = T(18, "rzd")
        V.reciprocal(rzd[:], zd[:])
        z = T(18, "z")      # (x-1)/(x+1) = 1 - 2/(x+1); x=inf -> z=1, no NaN
        V.tensor_scalar(z[:], rzd[:], -2.0, 1.0, A.mult, A.add)
        ats = T(18, "ats")
        S.activation(ats[:], z[:], AF.Arctan)

        # === Pool: independent work first (in-order engine; no stalls)
        b25 = T(1, "b25")
        G.memset(b25[:], 2.5)
        dwt = T(FREE, "dwt")    # 1 + 1.2*density
        G.tensor_scalar(dwt[:], dn, ALPHA_D, 1.0, A.mult, A.add)
        ar = T(18, "ar")        # a1|a2 = pw*ph | tw*th
        G.tensor_tensor(r2(ar[:]), w_in, h_in, A.mult)
        u0 = T(W, "u0")
        G.tensor_tensor(u0[:], ar[:, 0:W], ar[:, W:2 * W], A.add)
        u0e = T(W, "u0e")       # a1 + a2 + EPS
        G.tensor_scalar(u0e[:], u0[:], EPS, None, A.add)
        ad = T(FREE, "ad")
        G.tensor_scalar(ad[:], ar[:, W:W + FREE], 1e-7, None, A.add)
        dxy = T(18, "dxy")
        G.tensor_tensor(dxy[:], P2[:, 18:36], P2[:, 0:18], A.subtract)
        dsq = T(18, "dsq")
        G.tensor_tensor(dsq[:], dxy[:], dxy[:], A.mult)
        rho2 = T(W, "rho2")
        G.tensor_tensor(rho2[:], dsq[:, 0:W], dsq[:, W:2 * W], A.add)
        diff = T(D, "diff")
        diff_inst = G.tensor_tensor(diff[:], ei, ej, A.subtract)
        sq2 = T(D, "sq2")
        G.tensor_tensor(sq2[:], diff[:], diff[:], A.mult)

        # === DVE: corners / intersection / union / iou
        lo = T(36, "lo")        # b1x1|b1y1|b2x1|b2y1
        hi = T(36, "hi")        # b1x2|b1y2|b2x2|b2y2
        V.scalar_tensor_tensor(lo[:], WH, -0.5, P2, A.mult, A.add)
        V.scalar_tensor_tensor(hi[:], WH, 0.5, P2, A.mult, A.add)
        mlo = T(18, "mlo")
        mhi = T(18, "mhi")
        V.tensor_tensor(mlo[:], lo[:, 0:18], lo[:, 18:36], A.max)
        V.tensor_tensor(mhi[:], hi[:, 0:18], hi[:, 18:36], A.min)
        iwh = T(18, "iwh")      # unclipped intersection extents
        V.tensor_tensor(iwh[:], mhi[:], mlo[:], A.subtract)
        iwr = T(18, "iwr")
        V.tensor_scalar_max(iwr[:], iwh[:], 0.0)
        inter = T(W, "inter")
        V.tensor_tensor(inter[:], iwr[:, 0:W], iwr[:, W:2 * W], A.mult)
        u2 = T(W, "u2")         # union = u0e - inter
        V.scalar_tensor_tensor(u2[:], inter[:], -1.0, u0e[:], A.mult, A.add)
        ru = T(W, "ru")
        V.reciprocal(ru[:], u2[:])
        iou = T(W, "iou")
        V.tensor_tensor(iou[:], inter[:], ru[:], A.mult)
        c0 = T(18, "c0")
        c1 = T(18, "c1")
        V.tensor_tensor(c0[:], hi[:, 0:18], hi[:, 18:36], A.max)
        V.tensor_tensor(c1[:], lo[:, 0:18], lo[:, 18:36], A.min)
        stats = T(3, "stats")

        # === Pool: enclosing-box chain (c0/c1 handed off from DVE)
        cwh = T(18, "cwh")
        G.tensor_tensor(cwh[:], c0[:], c1[:], A.subtract)
        csq = T(18, "csq")
        G.tensor_tensor(csq[:], cwh[:], cwh[:], A.mult)
        c2 = T(W, "c2")
        c2_inst = G.tensor_tensor(c2[:], csq[:, 0:W], csq[:, W:2 * W], A.add)
        # Pool runs in order: embeddings diff must not delay the c-chain
        add_dep_helper(diff_inst.ins, c2_inst.ins, sync=False,
                       reason="order Pool diff after c2")
        mask = T(1, "mask")
        G.tensor_scalar(mask[:], iou[:, FREE:W], TAU, None, A.is_gt)

        # === DVE: alpha chain; v = VS*dv2 with VS = 4/pi^2 folded in
        VS = 4.0 / np.pi ** 2
        dv = T(W, "dv")
        V.tensor_tensor(dv[:], ats[:, W:2 * W], ats[:, 0:W], A.subtract)
        dv2 = T(W, "dv2")
        V.tensor_tensor(dv2[:], dv[:], dv[:], A.mult)
        d0 = T(W, "d0")         # v - iou
        V.scalar_tensor_tensor(d0[:], dv2[:], VS, iou[:], A.mult, A.subtract)
        d1 = T(W, "d1")
        V.tensor_scalar_add(d1[:], d0[:], 1.0 + EPS)
        rd = T(W, "rd")
        V.reciprocal(rd[:], d1[:])
        vv = T(W, "vv")         # dv2^2
        V.tensor_tensor(vv[:], dv2[:], dv2[:], A.mult)
        va = T(W, "va")         # v^2/d1 = v*alpha
        va_inst = V.scalar_tensor_tensor(va[:], vv[:], VS * VS, rd[:],
                                         A.mult, A.mult)
        c2e = T(W, "c2e")
        c2e_inst = V.tensor_scalar_add(c2e[:], c2[:], EPS)
        # keep the v/alpha chain ahead of the c-chain tail on DVE
        add_dep_helper(c2e_inst.ins, va_inst.ins, sync=False,
                       reason="order c2e after va on DVE")
        rc2 = T(W, "rc2")
        V.reciprocal(rc2[:], c2e[:])
        rr = T(W, "rr")         # rho2 / c2
        V.tensor_tensor(rr[:], rho2[:], rc2[:], A.mult)
        pen = T(W, "pen")
        V.tensor_tensor(pen[:], rr[:], va[:], A.add)
        ciou = T(W, "ciou")
        V.tensor_tensor(ciou[:], iou[:], pen[:], A.subtract)

        omd = T(W, "omd")       # cols 0:8 = 1-ciou, col 8 = |ei-ej|^2
        om_inst = V.tensor_scalar(omd[:, 0:FREE], ciou[:, 0:FREE], -1.0, 1.0,
                                  A.mult, A.add)
        d2_inst = V.tensor_reduce(omd[:, FREE:W], sq2[:], axis=AX.X, op=A.add)
        # keep the d2 reduce out of the alpha chain's way on DVE: without
        # this the scheduler hoists it and DVE stalls on Pool's sq2
        add_dep_helper(d2_inst.ins, om_inst.ins, sync=False,
                       reason="order d2-reduce after om on DVE")

        # === ACT tail: Sigmoid first (needs only ciou, so the sqrt-table
        # load right after it starts ~0.5us earlier than the reverse order;
        # the load overlaps the DVE hin/p25 tail work).
        hwt = T(FREE, "hwt")    # sigmoid(5*(0.5-ciou))
        sig_inst = S.activation(hwt[:], ciou[:, 0:FREE], AF.Sigmoid,
                                scale=-5.0, bias=b25[:])
        st = T(W, "st")         # sqrt(om) | dist
        sqrt_inst = S.activation(st[:], omd[:], AF.Sqrt)
        add_dep_helper(sqrt_inst.ins, sig_inst.ins, sync=False,
                       reason="sigmoid before sqrt on ACT")

        # === Pool tail (sq/p25 only need om/st; m1/scr wait for sigmoid)
        sq = T(FREE, "sq")
        G.tensor_tensor(sq[:], omd[:, 0:FREE], omd[:, 0:FREE], A.mult)
        p25 = T(FREE, "p25")    # (1-ciou)^2.5
        G.tensor_tensor(p25[:], sq[:], st[:, 0:FREE], A.mult)

        # === DVE tail (ia/reduce1 fill the table-load idle windows)
        ia = T(FREE, "ia")
        ia_inst = V.reciprocal(ia[:], ad[:])
        add_dep_helper(ia_inst.ins, om_inst.ins, sync=False,
                       reason="keep ia out of the pre-om DVE stream")
        V.tensor_reduce(stats[:, 1:2], ia[:], axis=AX.X, op=A.add)
        hin = T(1, "hin")       # relu(DELTA - dist)
        V.tensor_scalar(hin[:], st[:, FREE:W], -1.0, DELTA, A.mult, A.add)
        hinr = T(1, "hinr")
        V.tensor_scalar_max(hinr[:], hin[:], 0.0)
        h2 = T(1, "h2")
        V.tensor_tensor(h2[:], hinr[:], hinr[:], A.mult)
        m1 = T(FREE, "m1")
        V.tensor_tensor(m1[:], dwt[:], hwt[:], A.mult)
        scr = T(FREE, "scr")
        V.tensor_tensor(scr[:], m1[:], p25[:], A.mult)
        V.tensor_reduce(stats[:, 0:1], scr[:], axis=AX.X, op=A.add)
        V.tensor_tensor(stats[:, 2:3], mask[:], h2[:], A.mult)

        nc.sync.dma_start(out_d.ap(), stats[:])

    nc.compile()
    return nc


def _get_nc():
    global _BUILT
    if _BUILT is None:
        _BUILT = _build_nc()
    return _BUILT


def _pack_inputs(pred_boxes, target_boxes, embeddings, density_map, indices):
    pred = np.ascontiguousarray(pred_boxes, dtype=np.float32)
    targ = np.ascontiguousarray(target_boxes, dtype=np.float32)
    emb = np.ascontiguousarray(embeddings, dtype=np.float32)
    dens = np.ascontiguousarray(density_map, dtype=np.float32)
    idx = np.asarray(indices).astype(np.int64)

    i0, i1 = idx[:, 0], idx[:, 1]
    bi = np.ones((PPART, 4), np.float32)
    bj = np.ones((PPART, 4), np.float32)
    bi[:NPAIR] = pred[i0]
    bj[:NPAIR] = pred[i1]
    ei = np.zeros((PPART, D), np.float32)
    ej = np.zeros((PPART, D), np.float32)
    ei[:NPAIR] = emb[i0]
    ej[:NPAIR] = emb[i1]

    in_maps = []
    for c in range(N_CORES):
        s = slice(c * NS, (c + 1) * NS)
        pbs = pred[s].reshape(PPART, FREE, 4)
        tbs = targ[s].reshape(PPART, FREE, 4)
        buf = np.empty((PPART, 592), np.float32)
        # P2 blocks: px py tx ty ; WH blocks: pw ph tw th
        for k, (src, comp) in enumerate(
                [(pbs, 0), (pbs, 1), (tbs, 0), (tbs, 1),
                 (pbs, 2), (pbs, 3), (tbs, 2), (tbs, 3)]):
            pair = (bi if src is pbs else bj)[:, comp]
            buf[:, k * W:k * W + FREE] = src[:, :, comp]
            buf[:, k * W + FREE] = pair
        buf[:, 72:80] = dens[s].reshape(PPART, FREE)
        buf[:, 80:336] = ei
        buf[:, 336:592] = ej
        in_maps.append({"buf": buf})
    return in_maps


def kernel(pred_boxes, target_boxes, embeddings, density_map, indices):
    global LAST_RESULT
    import time as _time

    from concourse.bass_utils import run_bass_kernel_spmd

    nc = _get_nc()
    in_maps = _pack_inputs(pred_boxes, target_boxes, embeddings,
                           density_map, indices)
    for attempt in range(3):
        try:
            res = run_bass_kernel_spmd(nc, in_maps,
                                       core_ids=list(range(N_CORES)))
            break
        except Exception:
            # a crashed earlier run can leave a core wedged
            # (NRT_EXEC_UNIT_UNRECOVERABLE); it clears on retry
            if attempt == 2:
                raise
            _time.sleep(2.0)
    LAST_RESULT = res

    stats = np.stack([res.results[c]["out"] for c in range(N_CORES)])
    s_a = float(np.sum(stats[:, :, 0], dtype=np.float64))
    s_b = float(np.sum(stats[:, :, 1], dtype=np.float64))
    contrast = float(np.sum(stats[0, :NPAIR, 2], dtype=np.float64))
    loss = s_a * s_b / (N * N) + LAMBDA_C * contrast / (NPAIR + 1e-7)
    return np.asarray(np.float32(loss))

